# revision 10
# baseline (speedup 1.0000x reference)
"""Trainium2 Bass kernel: per-batch segment-mean pooling + 3-layer MLP.

Reference computation (B=64, T=512, H=768, S=128):
  pooled[b,s,:] = mean over t of hidden[b,t,:] where statements_ids[b,t]==s
  x = gelu(pooled @ w1 + b1); x = gelu(x @ w2 + b2)
  out[b,s] = sigmoid(x @ w3 + b3)

Distribution: data-parallel over batch across 8 NeuronCores (8 batches per
core); MLP weights replicated.

Per-core algorithm (all matmuls bf16, 1 cyc/row on PE):
  - mt_scaled[t,s] = (sid[t]==s)/count[sid[t]] built in one fused DVE
    tensor_scalar (is_equal then mult); inverse counts precomputed on host.
  - pooledT[h,s] = hidden_tile[t,h].T @ mt_scaled[t,s]  (PE, accumulated
    over the 4 t-tiles) -> already in the MLP's [contraction-on-partition]
    layout, so no PE transposes at all.
  - PSUM->SBUF drains of pooledT split between DVE and GPSIMD.
  - MLP: weights stationary, activations moving; gelu/sigmoid + bias fused
    on ACT.  All gelus complete before any sigmoid so the activation table
    loads exactly twice (dummy ACTs prefetch each table off the critical
    path).
"""

import os
import sys

sys.path.insert(0, "/opt/trn_rl_repo")

import numpy as np
import ml_dtypes

import concourse.bass as bass
import concourse.mybir as mybir
import concourse.tile as tile
from concourse import bacc, bass_utils

B, T, H, S = 64, 512, 768, 128
N_CORES = 8
BL = B // N_CORES  # local batches per core
P = 128
KT = T // P        # t-tiles per batch
KH = H // P        # h-tiles
R = BL * S         # MLP rows per core (= 1024)
RC = 256           # fc1 moving-dim sub-chunk (2 batches)
RW = 512           # ACT drain / fc2 / fc3 chunk (4 batches)

# cpf (f32 packed consts) column layout
C_IOTA = 0                  # [128, 128] iota along free dim
C_SID = C_IOTA + P          # [128, 32]  sid, token-major per (b, k) column
C_ICNT = C_SID + BL * KT    # [128, 32]  1/count per token, same layout
C_B1 = C_ICNT + BL * KT     # [128, 6]
C_B2 = C_B1 + KH            # [128, 6]
C_B3 = C_B2 + KH            # [1] at row 0
CPF_COLS = C_B3 + 1

_CACHE: dict = {}


def _build_program():
    f32, bf16 = mybir.dt.float32, mybir.dt.bfloat16
    FT = mybir.ActivationFunctionType
    OP = mybir.AluOpType

    nc = bacc.Bacc("TRN2", target_bir_lowering=False, debug=False)
    hid = nc.dram_tensor("hidden", [BL, T, H], bf16, kind="ExternalInput").ap()
    w1 = nc.dram_tensor("w1", [H, H], bf16, kind="ExternalInput").ap()
    w2 = nc.dram_tensor("w2", [H, H], bf16, kind="ExternalInput").ap()
    w3p = nc.dram_tensor("w3p", [P, KH], bf16, kind="ExternalInput").ap()
    cpf = nc.dram_tensor("cpf", [P, CPF_COLS], f32, kind="ExternalInput").ap()
    out = nc.dram_tensor("out", [BL, S], f32, kind="ExternalOutput").ap()

    with tile.TileContext(nc) as tc:
        with (
            tc.tile_pool(name="consts", bufs=1) as consts,
            tc.tile_pool(name="wpool", bufs=1) as wpool,
            tc.tile_pool(name="hpool", bufs=1) as hpool,
            tc.tile_pool(name="mtpool", bufs=8) as mtpool,
            tc.tile_pool(name="xpool", bufs=1) as xpool,
            tc.tile_pool(name="pp", bufs=4, space="PSUM") as pp,
            tc.tile_pool(name="pf", bufs=2, space="PSUM") as pf,
            tc.tile_pool(name="p3", bufs=1, space="PSUM") as p3,
        ):
            # consts first on the sync ring so mt builds unlock immediately
            cpf_sb = consts.tile([P, CPF_COLS], f32)
            nc.sync.dma_start(cpf_sb, cpf)
            iota = cpf_sb[:, C_IOTA:C_IOTA + P]
            sidf = cpf_sb[:, C_SID:C_SID + BL * KT]
            icnt = cpf_sb[:, C_ICNT:C_ICNT + BL * KT]

            # all input DMAs share the sync ring so arrival order is exactly
            # the priority order: cpf, hb0 (per k-tile), w1, hb1-hb3, w2,
            # hb4-hb7, w3 (only needed for fc3, ~60us in)
            hbs = [None] * BL

            def load_hb(b):
                if b < 2:
                    tiles = []
                    for k in range(KT):
                        t_ = hpool.tile([P, H], bf16, tag=f"hb{b}k{k}")
                        nc.sync.dma_start(t_, hid[b, k * P:(k + 1) * P, :])
                        tiles.append(t_)
                    hbs[b] = tiles
                else:
                    hb = hpool.tile(
                        [P, KT, H], bf16, tag=f"hbr{(b - 2) % 4}", name=f"hb{b}"
                    )
                    nc.sync.dma_start(hb, hid[b].rearrange("(k p) h -> p k h", p=P))
                    hbs[b] = hb

            def hsl(b, k, h):
                if b < 2:
                    return hbs[b][k][:, h * P:(h + 1) * P]
                return hbs[b][:, k, h * P:(h + 1) * P]

            load_hb(0)
            w1sb = wpool.tile([P, KH, H], bf16)
            nc.sync.dma_start(w1sb, w1.rearrange("(k p) h -> p k h", p=P))
            load_hb(1)
            load_hb(2)
            load_hb(3)
            w2sb = wpool.tile([P, KH, H], bf16)
            nc.sync.dma_start(w2sb, w2.rearrange("(k p) h -> p k h", p=P))
            load_hb(4)
            load_hb(5)
            load_hb(6)
            load_hb(7)
            w3sb = wpool.tile([P, KH], bf16)
            nc.sync.dma_start(w3sb, w3p)

            xts = [xpool.tile([P, R], bf16, tag=f"xt{h}", name=f"xt{h}") for h in range(KH)]
            y1s = [xpool.tile([P, R], bf16, tag=f"y1_{m}", name=f"y1_{m}") for m in range(KH)]
            y2s = [xpool.tile([P, R], bf16, tag=f"y2_{m}", name=f"y2_{m}") for m in range(KH)]
            pred = xpool.tile([1, R], f32, tag="pred")
            scratch = xpool.tile([1, 1], f32, tag="scratch")

            # prefetch the gelu activation table during the DMA lead-in
            nc.scalar.activation(scratch, cpf_sb[0:1, 0:1], FT.Gelu)

            def pool(b):
                mts = []
                for k in range(KT):
                    c = b * KT + k
                    mt = mtpool.tile([P, P], bf16, tag="mt")
                    nc.gpsimd.tensor_scalar(
                        mt, iota, sidf[:, c:c + 1], icnt[:, c:c + 1],
                        OP.is_equal, OP.mult,
                    )
                    mts.append(mt)
                for h in range(KH):
                    pt = pp.tile([P, P], f32, tag="pp", name=f"pp{b}_{h}")
                    for k in range(KT):
                        nc.tensor.matmul(
                            pt, lhsT=hsl(b, k, h), rhs=mts[k],
                            start=(k == 0), stop=(k == KT - 1),
                        )
                    nc.vector.tensor_copy(xts[h][:, b * S:(b + 1) * S], pt)

            def fc1(rcp):
                for m in range(KH):
                    pt = pf.tile([P, RW], f32, tag="pf")
                    for half in range(2):
                        rc = 2 * rcp + half
                        for k in range(KH):
                            nc.tensor.matmul(
                                pt[:, half * RC:(half + 1) * RC],
                                lhsT=w1sb[:, k, m * P:(m + 1) * P],
                                rhs=xts[k][:, rc * RC:(rc + 1) * RC],
                                start=(k == 0), stop=(k == KH - 1),
                            )
                    nc.scalar.activation(
                        y1s[m][:, rcp * RW:(rcp + 1) * RW], pt, FT.Gelu,
                        bias=cpf_sb[:, C_B1 + m:C_B1 + m + 1],
                    )

            def fc2(rcp):
                for m in range(KH):
                    pt = pf.tile([P, RW], f32, tag="pf")
                    for k in range(KH):
                        nc.tensor.matmul(
                            pt,
                            lhsT=w2sb[:, k, m * P:(m + 1) * P],
                            rhs=y1s[k][:, rcp * RW:(rcp + 1) * RW],
                            start=(k == 0), stop=(k == KH - 1),
                        )
                    nc.scalar.activation(
                        y2s[m][:, rcp * RW:(rcp + 1) * RW], pt, FT.Gelu,
                        bias=cpf_sb[:, C_B2 + m:C_B2 + m + 1],
                    )

            p3s = [None, None]

            def fc3(rcp):
                pt3 = p3.tile([1, RW], f32, tag=f"p3{rcp}")
                for k in range(KH):
                    nc.tensor.matmul(
                        pt3,
                        lhsT=w3sb[:, k:k + 1],
                        rhs=y2s[k][:, rcp * RW:(rcp + 1) * RW],
                        start=(k == 0), stop=(k == KH - 1),
                    )
                p3s[rcp] = pt3

            for b in range(4):
                pool(b)
            fc1(0)
            for b in range(4, BL):
                pool(b)
            fc1(1)
            fc2(0)
            fc3(0)
            fc2(1)
            nc.scalar.activation(
                pred[:, 0:RW], p3s[0], FT.Sigmoid, bias=cpf_sb[0:1, C_B3:C_B3 + 1]
            )
            fc3(1)
            nc.scalar.activation(
                pred[:, RW:2 * RW], p3s[1], FT.Sigmoid,
                bias=cpf_sb[0:1, C_B3:C_B3 + 1],
            )
            nc.sync.dma_start(out.rearrange("b s -> (b s)"), pred[:, :])

    nc.compile()
    return nc


def _get_program():
    if "nc" not in _CACHE:
        _CACHE["nc"] = _build_program()
    return _CACHE["nc"]


def _tok_cols(x):
    """[BL, T] -> [128, BL*KT], column c=b*KT+k holds tokens k*128..k*128+127."""
    return np.transpose(x.reshape(BL, KT, P), (2, 0, 1)).reshape(P, BL * KT)


def make_in_maps(hidden, statements_ids, w1, b1, w2, b2, w3, b3):
    bf = ml_dtypes.bfloat16
    hid_b = np.ascontiguousarray(np.asarray(hidden, np.float32).astype(bf))
    w1b = np.ascontiguousarray(np.asarray(w1, np.float32).astype(bf))
    w2b = np.ascontiguousarray(np.asarray(w2, np.float32).astype(bf))
    w3v = np.asarray(w3, np.float32).reshape(H)
    w3pk = np.ascontiguousarray(w3v.reshape(KH, P).T.astype(bf))  # [128, 6]
    sid = np.asarray(statements_ids, np.int32)
    # per-token inverse segment count (count >= 1 for a token's own sid)
    cnt = (sid[:, :, None] == np.arange(S)[None, None, :]).sum(1)  # [B, S]
    icnt_tok = (1.0 / np.take_along_axis(cnt, sid, 1)).astype(np.float32)

    b1v = np.asarray(b1, np.float32).reshape(KH, P).T  # [128, 6]
    b2v = np.asarray(b2, np.float32).reshape(KH, P).T
    b3v = np.float32(np.asarray(b3).reshape(-1)[0])

    in_maps = []
    for c in range(N_CORES):
        lo, hi = c * BL, (c + 1) * BL
        cpf = np.zeros((P, CPF_COLS), dtype=np.float32)
        cpf[:, C_IOTA:C_IOTA + P] = np.arange(P, dtype=np.float32)[None, :]
        cpf[:, C_SID:C_SID + BL * KT] = _tok_cols(sid[lo:hi].astype(np.float32))
        cpf[:, C_ICNT:C_ICNT + BL * KT] = _tok_cols(icnt_tok[lo:hi])
        cpf[:, C_B1:C_B1 + KH] = b1v
        cpf[:, C_B2:C_B2 + KH] = b2v
        cpf[0, C_B3] = b3v
        in_maps.append(
            {
                "hidden": hid_b[lo:hi],
                "w1": w1b,
                "w2": w2b,
                "w3p": w3pk,
                "cpf": cpf,
            }
        )
    return in_maps


def kernel(hidden, statements_ids, w1, b1, w2, b2, w3, b3, **kwargs):
    nc = _get_program()
    in_maps = make_in_maps(hidden, statements_ids, w1, b1, w2, b2, w3, b3)
    trace = bool(int(os.environ.get("KERNEL_TRACE", "0")))
    res = bass_utils.run_bass_kernel_spmd(
        nc, in_maps, core_ids=list(range(N_CORES)), trace=trace
    )
    _CACHE["last_results"] = res
    out = np.concatenate([res.results[c]["out"] for c in range(N_CORES)], axis=0)
    return out.astype(np.float32)


# revision 11
# speedup vs baseline: 1.5686x; 1.5686x over previous
"""Trainium2 Bass kernel: per-batch segment-mean pooling + 3-layer MLP.

Reference computation (B=64, T=512, H=768, S=128):
  pooled[b,s,:] = mean over t of hidden[b,t,:] where statements_ids[b,t]==s
  x = gelu(pooled @ w1 + b1); x = gelu(x @ w2 + b2)
  out[b,s] = sigmoid(x @ w3 + b3)

Distribution: data-parallel over batch across 8 NeuronCores (8 batches per
core); MLP weights replicated.

Per-core algorithm (all matmuls bf16, 1 cyc/row on PE):
  - mt_scaled[t,s] = (sid[t]==s)/count[sid[t]] built in one fused DVE
    tensor_scalar (is_equal then mult); inverse counts precomputed on host.
  - pooledT[h,s] = hidden_tile[t,h].T @ mt_scaled[t,s]  (PE, accumulated
    over the 4 t-tiles) -> already in the MLP's [contraction-on-partition]
    layout, so no PE transposes at all.
  - PSUM->SBUF drains of pooledT split between DVE and GPSIMD.
  - MLP: weights stationary, activations moving; gelu/sigmoid + bias fused
    on ACT.  All gelus complete before any sigmoid so the activation table
    loads exactly twice (dummy ACTs prefetch each table off the critical
    path).
"""

import os
import sys

sys.path.insert(0, "/opt/trn_rl_repo")

import numpy as np
import ml_dtypes

import concourse.bass as bass
import concourse.mybir as mybir
import concourse.tile as tile
from concourse import bacc, bass_utils

B, T, H, S = 64, 512, 768, 128
N_CORES = 8
BL = B // N_CORES  # local batches per core
P = 128
KT = T // P        # t-tiles per batch
KH = H // P        # h-tiles
R = BL * S         # MLP rows per core (= 1024)
RC = 256           # fc1 moving-dim sub-chunk (2 batches)
RW = 512           # ACT drain / fc2 / fc3 chunk (4 batches)

# cpf (f32 packed consts) column layout
C_IOTA = 0                  # [128, 128] iota along free dim
C_SID = C_IOTA + P          # [128, 32]  sid, token-major per (b, k) column
C_ICNT = C_SID + BL * KT    # [128, 32]  1/count per token, same layout
C_B1 = C_ICNT + BL * KT     # [128, 6]
C_B2 = C_B1 + KH            # [128, 6]
C_B3 = C_B2 + KH            # [1] at row 0
CPF_COLS = C_B3 + 1

_CACHE: dict = {}


def _build_program():
    f32, bf16 = mybir.dt.float32, mybir.dt.bfloat16
    FT = mybir.ActivationFunctionType
    OP = mybir.AluOpType

    nc = bacc.Bacc("TRN2", target_bir_lowering=False, debug=False)
    hid = nc.dram_tensor("hidden", [BL, T, H], bf16, kind="ExternalInput").ap()
    w1 = nc.dram_tensor("w1", [H, H], bf16, kind="ExternalInput").ap()
    w2 = nc.dram_tensor("w2", [H, H], bf16, kind="ExternalInput").ap()
    w3p = nc.dram_tensor("w3p", [P, KH], bf16, kind="ExternalInput").ap()
    cpf = nc.dram_tensor("cpf", [P, CPF_COLS], f32, kind="ExternalInput").ap()
    out = nc.dram_tensor("out", [BL, S], f32, kind="ExternalOutput").ap()

    with tile.TileContext(nc) as tc:
        with (
            tc.tile_pool(name="consts", bufs=1) as consts,
            tc.tile_pool(name="wpool", bufs=1) as wpool,
            tc.tile_pool(name="hpool", bufs=1) as hpool,
            tc.tile_pool(name="mtpool", bufs=8) as mtpool,
            tc.tile_pool(name="xpool", bufs=1) as xpool,
            tc.tile_pool(name="pp", bufs=4, space="PSUM") as pp,
            tc.tile_pool(name="pf", bufs=2, space="PSUM") as pf,
            tc.tile_pool(name="p3", bufs=1, space="PSUM") as p3,
        ):
            # consts first on the sync ring so mt builds unlock immediately
            cpf_sb = consts.tile([P, CPF_COLS], f32)
            nc.sync.dma_start(cpf_sb, cpf)
            iota = cpf_sb[:, C_IOTA:C_IOTA + P]
            sidf = cpf_sb[:, C_SID:C_SID + BL * KT]
            icnt = cpf_sb[:, C_ICNT:C_ICNT + BL * KT]

            # all input DMAs share the sync ring so arrival order is exactly
            # the priority order: cpf, hb0 (per k-tile), w1, hb1-hb3, w2,
            # hb4-hb7, w3 (only needed for fc3, ~60us in)
            hbs = [None] * BL

            def load_hb(b):
                if b < 2:
                    tiles = []
                    for k in range(KT):
                        t_ = hpool.tile([P, H], bf16, tag=f"hb{b}k{k}")
                        nc.sync.dma_start(t_, hid[b, k * P:(k + 1) * P, :])
                        tiles.append(t_)
                    hbs[b] = tiles
                else:
                    hb = hpool.tile(
                        [P, KT, H], bf16, tag=f"hbr{(b - 2) % 4}", name=f"hb{b}"
                    )
                    nc.sync.dma_start(hb, hid[b].rearrange("(k p) h -> p k h", p=P))
                    hbs[b] = hb

            def hsl(b, k, h):
                if b < 2:
                    return hbs[b][k][:, h * P:(h + 1) * P]
                return hbs[b][:, k, h * P:(h + 1) * P]

            load_hb(0)
            w1sb = wpool.tile([P, KH, H], bf16)
            nc.sync.dma_start(w1sb, w1.rearrange("(k p) h -> p k h", p=P))
            load_hb(1)
            load_hb(2)
            load_hb(3)
            w2sb = wpool.tile([P, KH, H], bf16)
            nc.sync.dma_start(w2sb, w2.rearrange("(k p) h -> p k h", p=P))
            load_hb(4)
            load_hb(5)
            load_hb(6)
            load_hb(7)
            w3sb = wpool.tile([P, KH], bf16)
            nc.sync.dma_start(w3sb, w3p)

            xts = [xpool.tile([P, R], bf16, tag=f"xt{h}", name=f"xt{h}") for h in range(KH)]
            y1s = [xpool.tile([P, R], bf16, tag=f"y1_{m}", name=f"y1_{m}") for m in range(KH)]
            y2s = [xpool.tile([P, R], bf16, tag=f"y2_{m}", name=f"y2_{m}") for m in range(KH)]
            pred = xpool.tile([1, R], f32, tag="pred")
            scratch = xpool.tile([1, 1], f32, tag="scratch")

            # prefetch the gelu activation table during the DMA lead-in
            nc.scalar.activation(scratch, cpf_sb[0:1, 0:1], FT.Gelu)

            def pool(b):
                mts = []
                for k in range(KT):
                    c = b * KT + k
                    mt = mtpool.tile([P, P], bf16, tag="mt")
                    nc.vector.tensor_scalar(
                        mt, iota, sidf[:, c:c + 1], icnt[:, c:c + 1],
                        OP.is_equal, OP.mult,
                    )
                    mts.append(mt)
                for h in range(KH):
                    pt = pp.tile([P, P], f32, tag="pp", name=f"pp{b}_{h}")
                    for k in range(KT):
                        nc.tensor.matmul(
                            pt, lhsT=hsl(b, k, h), rhs=mts[k],
                            start=(k == 0), stop=(k == KT - 1),
                        )
                    nc.vector.tensor_copy(xts[h][:, b * S:(b + 1) * S], pt)

            def fc1(rcp):
                for m in range(KH):
                    pt = pf.tile([P, RW], f32, tag="pf")
                    for half in range(2):
                        rc = 2 * rcp + half
                        for k in range(KH):
                            nc.tensor.matmul(
                                pt[:, half * RC:(half + 1) * RC],
                                lhsT=w1sb[:, k, m * P:(m + 1) * P],
                                rhs=xts[k][:, rc * RC:(rc + 1) * RC],
                                start=(k == 0), stop=(k == KH - 1),
                            )
                    nc.scalar.activation(
                        y1s[m][:, rcp * RW:(rcp + 1) * RW], pt, FT.Gelu,
                        bias=cpf_sb[:, C_B1 + m:C_B1 + m + 1],
                    )

            def fc2(rcp):
                for m in range(KH):
                    pt = pf.tile([P, RW], f32, tag="pf")
                    for k in range(KH):
                        nc.tensor.matmul(
                            pt,
                            lhsT=w2sb[:, k, m * P:(m + 1) * P],
                            rhs=y1s[k][:, rcp * RW:(rcp + 1) * RW],
                            start=(k == 0), stop=(k == KH - 1),
                        )
                    nc.scalar.activation(
                        y2s[m][:, rcp * RW:(rcp + 1) * RW], pt, FT.Gelu,
                        bias=cpf_sb[:, C_B2 + m:C_B2 + m + 1],
                    )

            p3s = [None, None]

            def fc3(rcp):
                pt3 = p3.tile([1, RW], f32, tag=f"p3{rcp}")
                for k in range(KH):
                    nc.tensor.matmul(
                        pt3,
                        lhsT=w3sb[:, k:k + 1],
                        rhs=y2s[k][:, rcp * RW:(rcp + 1) * RW],
                        start=(k == 0), stop=(k == KH - 1),
                    )
                p3s[rcp] = pt3

            for b in range(4):
                pool(b)
            fc1(0)
            for b in range(4, BL):
                pool(b)
            fc1(1)
            fc2(0)
            fc3(0)
            fc2(1)
            nc.scalar.activation(
                pred[:, 0:RW], p3s[0], FT.Sigmoid, bias=cpf_sb[0:1, C_B3:C_B3 + 1]
            )
            fc3(1)
            nc.scalar.activation(
                pred[:, RW:2 * RW], p3s[1], FT.Sigmoid,
                bias=cpf_sb[0:1, C_B3:C_B3 + 1],
            )
            nc.sync.dma_start(out.rearrange("b s -> (b s)"), pred[:, :])

    nc.compile()
    return nc


def _get_program():
    if "nc" not in _CACHE:
        _CACHE["nc"] = _build_program()
    return _CACHE["nc"]


def _tok_cols(x):
    """[BL, T] -> [128, BL*KT], column c=b*KT+k holds tokens k*128..k*128+127."""
    return np.transpose(x.reshape(BL, KT, P), (2, 0, 1)).reshape(P, BL * KT)


def make_in_maps(hidden, statements_ids, w1, b1, w2, b2, w3, b3):
    bf = ml_dtypes.bfloat16
    hid_b = np.ascontiguousarray(np.asarray(hidden, np.float32).astype(bf))
    w1b = np.ascontiguousarray(np.asarray(w1, np.float32).astype(bf))
    w2b = np.ascontiguousarray(np.asarray(w2, np.float32).astype(bf))
    w3v = np.asarray(w3, np.float32).reshape(H)
    w3pk = np.ascontiguousarray(w3v.reshape(KH, P).T.astype(bf))  # [128, 6]
    sid = np.asarray(statements_ids, np.int32)
    # per-token inverse segment count (count >= 1 for a token's own sid)
    cnt = (sid[:, :, None] == np.arange(S)[None, None, :]).sum(1)  # [B, S]
    icnt_tok = (1.0 / np.take_along_axis(cnt, sid, 1)).astype(np.float32)

    b1v = np.asarray(b1, np.float32).reshape(KH, P).T  # [128, 6]
    b2v = np.asarray(b2, np.float32).reshape(KH, P).T
    b3v = np.float32(np.asarray(b3).reshape(-1)[0])

    in_maps = []
    for c in range(N_CORES):
        lo, hi = c * BL, (c + 1) * BL
        cpf = np.zeros((P, CPF_COLS), dtype=np.float32)
        cpf[:, C_IOTA:C_IOTA + P] = np.arange(P, dtype=np.float32)[None, :]
        cpf[:, C_SID:C_SID + BL * KT] = _tok_cols(sid[lo:hi].astype(np.float32))
        cpf[:, C_ICNT:C_ICNT + BL * KT] = _tok_cols(icnt_tok[lo:hi])
        cpf[:, C_B1:C_B1 + KH] = b1v
        cpf[:, C_B2:C_B2 + KH] = b2v
        cpf[0, C_B3] = b3v
        in_maps.append(
            {
                "hidden": hid_b[lo:hi],
                "w1": w1b,
                "w2": w2b,
                "w3p": w3pk,
                "cpf": cpf,
            }
        )
    return in_maps


def kernel(hidden, statements_ids, w1, b1, w2, b2, w3, b3, **kwargs):
    nc = _get_program()
    in_maps = make_in_maps(hidden, statements_ids, w1, b1, w2, b2, w3, b3)
    trace = bool(int(os.environ.get("KERNEL_TRACE", "0")))
    res = bass_utils.run_bass_kernel_spmd(
        nc, in_maps, core_ids=list(range(N_CORES)), trace=trace
    )
    _CACHE["last_results"] = res
    out = np.concatenate([res.results[c]["out"] for c in range(N_CORES)], axis=0)
    return out.astype(np.float32)
